# revision 4
# baseline (speedup 1.0000x reference)
"""CPDecoding (embedding_lookup) Trainium2 kernel, v3.

out[n] = sum_c fz[c,n]*fy[c,n]*fx[c,n], each f* a 1-D linear interpolation
(grid_sample, align_corners=True) of a (96, 512) line table at per-point
coordinates in [0,1).

Strategy (8 cores, data-parallel over N=4096*192 points):
  - Host: shard points; SORT each shard by z-position so consecutive points
    share z table rows; pack coordinates into gather-layouts; build
      * tblz: coarse z table [512, 256] fp16 rows = [f0(96) | delta(96) |
        row_idx | pad] (512B rows),
      * tbly/tblx: fine pre-interpolated tables [32768, 128] fp16 where row
        j = interp(L, (j+32704)/128) (Q=128 sub-steps, 256B rows).
  - Device: octets of 8 z-sorted points share ONE 512B z-row gather
    (8x descriptor sharing); per-point 256B y/x gathers; exact z interp
    fz = f0 + (posz - row_idx)*delta on DVE; fp16 triple product (one mul
    on the Pool engine); pairwise-tree component reduction.
  - Host: inverse-permute the per-core outputs back to input order.

Quantization error (y/x at Q=128 + fp16): rel err ~5.8e-3 (gate 2e-2).
"""

import numpy as np

N_CORES = 8
N_TOTAL = 4096 * 192
N_CORE = N_TOTAL // N_CORES      # 98304 points per core
P = 128                          # partitions
F = N_CORE // P                  # 768 f-columns
C = 96                           # components
R = 512                          # coarse table resolution
Q = 128                          # fine sub-steps per coarse cell (y/x)
SC = (R - 1) / 2 * Q             # 32704: j = round(coord * SC)
NJ = int(SC) + 1                 # 32705 used fine rows
NJ_PAD = 32768                   # padded fine-table rows
EY = 128                         # y/x gather row elems (fp16) = 256B
EZ = 256                         # z gather row elems (fp16) = 512B
GROUPS = 8                       # staging groups (16-partition bands)
CHUNKS_PER_GROUP = 3
N_CHUNKS = GROUPS * CHUNKS_PER_GROUP          # 24
CHUNK_F = F // N_CHUNKS                       # 32 f-cols per chunk
CHUNK_PTS = P * CHUNK_F                       # 4096 points per chunk
OCT = 8                                       # points per shared z-row
OBLK = CHUNK_F // OCT                         # 4 octet blocks per chunk
OCT_G = CHUNKS_PER_GROUP * CHUNK_PTS // OCT   # 1536 octets per group
# combined int16 idx tile columns: [jy (768) | jx (768) | zi (96)]
JY0, JX0, ZI0 = 0, F, 2 * F
JCOLS = 2 * F + F // OCT         # 1632
# combined fp32 input columns: [zc (768) | y16 (768) | x16 (768) | zo16 (96)]
ZC0, Y0, X0, ZO0 = 0, F, 2 * F, 3 * F
PWCOLS = 3 * F + F // OCT        # 2400

_BUILT = None
_MAPS = None


def _build_static_maps():
    """Static slot->rank index maps (no data dependence).

    Processing slot of chunk c: s in [0,4096) -> [p=s%128, f=32c+s//128].
    Octet grouping: df = s//128 = 8m+u; octet o = m*128+p holds sorted ranks
    r = c*4096 + o*8 + u (u=0..7 consecutive in z-sorted order).
    """
    p = np.arange(P)[:, None]
    f = np.arange(F)[None, :]
    c = f // CHUNK_F
    df = f % CHUNK_F
    m = df // OCT
    u = df % OCT
    rank_pf = c * CHUNK_PTS + (m * P + p) * OCT + u          # [128, 768]

    g = np.arange(GROUPS)[:, None, None]
    t = np.arange(16)[None, :, None]
    phi = np.arange(F)[None, None, :]
    sub = phi // 256
    s = (phi % 256) * 16 + t
    pp = s % P
    dff = s // P
    mm = dff // OCT
    uu = dff % OCT
    rank_y16 = ((3 * g + sub) * CHUNK_PTS + (mm * P + pp) * OCT + uu
                ).reshape(GROUPS * 16, F)                     # [128, 768]

    phio = np.arange(F // OCT)[None, None, :]
    og = phio * 16 + t                                        # octet-in-group
    subo = og // (CHUNK_PTS // OCT)
    rem = og % (CHUNK_PTS // OCT)
    rank_zo = ((3 * g + subo) * CHUNK_PTS + rem * OCT + 3
               ).reshape(GROUPS * 16, F // OCT)               # [128, 96]
    return rank_pf, rank_y16, rank_zo


def _build_nc():
    """Build the per-core Bass program (SPMD, identical on all cores)."""
    import concourse.bacc as bacc
    import concourse.tile as tile
    from concourse import mybir
    from concourse.library_config import mlp as lib_mlp

    dt = mybir.dt
    Alu = mybir.AluOpType
    Axis = mybir.AxisListType

    nc = bacc.Bacc("TRN2", target_bir_lowering=False, debug=False,
                   num_devices=N_CORES, num_swdge_queues=1)

    pwa = nc.dram_tensor("pwa", [P, PWCOLS], dt.float32,
                         kind="ExternalInput").ap()
    tblz = nc.dram_tensor("tblz", [R, EZ], dt.float16,
                          kind="ExternalInput").ap()
    tbly = nc.dram_tensor("tbly", [NJ_PAD, EY], dt.float16,
                          kind="ExternalInput").ap()
    tblx = nc.dram_tensor("tblx", [NJ_PAD, EY], dt.float16,
                          kind="ExternalInput").ap()
    out_d = nc.dram_tensor("out", [P, F], dt.float32,
                           kind="ExternalOutput").ap()

    with tile.TileContext(nc) as tc:
        with tc.tile_pool(name="persist", bufs=1) as pp:
            posz = pp.tile([P, F], dt.float32, tag="posz")
            jall = pp.tile([P, JCOLS], dt.int16, tag="jall")
            out_full = pp.tile([P, F], dt.float32, tag="out")

            # ---------- setup: load coords, index math ----------
            with tc.tile_pool(name="setup", bufs=1) as sp:
                pw = sp.tile([P, PWCOLS], dt.float32, tag="pw")
                nc.sync.dma_start(pw[:], pwa)

                # posz = zc*255.5 + 255.5  (exact coarse position, fp32)
                nc.vector.tensor_scalar(posz[:], pw[:, ZC0:ZC0 + F],
                                        255.5, 255.5, Alu.mult, Alu.add)

                def tmp(nm, ncols, dtype=dt.float32):
                    return sp.tile([P, ncols], dtype, tag="tmp", bufs=6,
                                   name=nm)

                # jy/jx = floor(y*SC + 0.5); explicit floor fixup so the
                # result is identical whether the fp->int cast truncates
                # (CoreSim) or rounds to nearest (hardware).
                for (src0, dstc, nm) in ((Y0, JY0, "jy"), (X0, JX0, "jx")):
                    jf = tmp(nm, F)
                    nc.vector.tensor_scalar(jf[:], pw[:, src0:src0 + F],
                                            float(SC), 0.5, Alu.mult, Alu.add)
                    ji = tmp(nm + "i", F, dt.int32)
                    nc.vector.tensor_copy(ji[:], jf[:])
                    jb = tmp(nm + "b", F)
                    nc.vector.tensor_copy(jb[:], ji[:])
                    jn = tmp(nm + "n", F)
                    nc.vector.tensor_tensor(jn[:], jf[:], jb[:], Alu.is_lt)
                    jg = tmp(nm + "g", F)
                    nc.vector.tensor_sub(jg[:], jb[:], jn[:])
                    nc.vector.tensor_copy(jall[:, dstc:dstc + F], jg[:])

                # zi = floor(zo*255.5 + 255.5) with floor fixup, clamp
                nzo = F // OCT
                zposf = tmp("zpos", nzo)
                nc.vector.tensor_scalar(zposf[:], pw[:, ZO0:ZO0 + nzo],
                                        255.5, 255.5, Alu.mult, Alu.add)
                zii = tmp("zii", nzo, dt.int32)
                nc.vector.tensor_copy(zii[:], zposf[:])
                zif = tmp("zif", nzo)
                nc.vector.tensor_copy(zif[:], zii[:])
                zneg = tmp("zneg", nzo)
                nc.vector.tensor_tensor(zneg[:], zposf[:], zif[:], Alu.is_lt)
                zfl = tmp("zfl", nzo)
                nc.vector.tensor_sub(zfl[:], zif[:], zneg[:])
                zcl = tmp("zcl", nzo)
                nc.vector.tensor_scalar(zcl[:], zfl[:], 511.0, 0.0,
                                        Alu.min, Alu.max)
                nc.vector.tensor_copy(jall[:, ZI0:ZI0 + nzo], zcl[:])

            # ---------- main loop ----------
            with (
                tc.tile_pool(name="stg", bufs=2) as stg_pool,
                tc.tile_pool(name="zg", bufs=2) as zg_pool,
                tc.tile_pool(name="gath", bufs=3) as gath_pool,
                tc.tile_pool(name="mid", bufs=3) as mid_pool,
            ):
                with tc.tile_critical():
                    nc.gpsimd.load_library(lib_mlp)

                for g in range(GROUPS):
                    # replicate group g's idx rows into every 16-part band
                    stg = stg_pool.tile([P, JCOLS], dt.int16, tag="stg")
                    src = jall[16 * g:16 * (g + 1), :]
                    for b in range(8):
                        nc.sync.dma_start(stg[16 * b:16 * (b + 1), :], src)

                    # one z-gather per group: 1536 octet rows of 512B
                    zd = zg_pool.tile([P, OCT_G // P, EZ], dt.float16,
                                      tag="zd")
                    nc.gpsimd.dma_gather(
                        zd[:], tblz, stg[:, ZI0:ZI0 + nzo], OCT_G, OCT_G,
                        EZ, elem_step=EZ, queue_num=0, single_packet=False)

                    for sub in range(CHUNKS_PER_GROUP):
                        c = CHUNKS_PER_GROUP * g + sub
                        gath = []
                        for (tb, col0, nm) in ((tbly, JY0, "y"),
                                               (tblx, JX0, "x")):
                            gt = gath_pool.tile([P, CHUNK_F, EY], dt.float16,
                                                tag=f"g{nm}")
                            idxs = stg[:, col0 + 256 * sub:col0 + 256 * (sub + 1)]
                            nc.gpsimd.dma_gather(
                                gt[:], tb, idxs, CHUNK_PTS, CHUNK_PTS, EY,
                                elem_step=EY, queue_num=0, single_packet=False)
                            gath.append(gt)

                        # g2 = fy * fx
                        g2 = mid_pool.tile([P, CHUNK_F, C], dt.float16,
                                           tag="g2")
                        nc.vector.tensor_mul(g2[:], gath[0][:, :, 0:C],
                                             gath[1][:, :, 0:C])

                        # wz = posz - row_idx (row idx baked in z-row elem 192)
                        zrow = zd[:, OBLK * sub:OBLK * (sub + 1), :]
                        i0ap = (zrow[:, :, 2 * C:2 * C + 1]
                                .broadcast_to([P, OBLK, OCT]))
                        pz = (posz[:, CHUNK_F * c:CHUNK_F * (c + 1)]
                              .rearrange("p (m u) -> p m u", u=OCT))
                        wz = mid_pool.tile([P, OBLK, OCT], dt.float16,
                                           tag="wz")
                        nc.vector.tensor_sub(wz[:], pz, i0ap)

                        # fz = f0 + wz*delta
                        wzb = wz[:].unsqueeze(3).broadcast_to(
                            [P, OBLK, OCT, C])
                        dzb = (zrow[:, :, C:2 * C].unsqueeze(2)
                               .broadcast_to([P, OBLK, OCT, C]))
                        f0b = (zrow[:, :, 0:C].unsqueeze(2)
                               .broadcast_to([P, OBLK, OCT, C]))
                        u1 = mid_pool.tile([P, CHUNK_F, C], dt.float16,
                                           tag="u1")
                        u1v = u1[:].rearrange("p (m u) e -> p m u e", u=OCT)
                        nc.vector.tensor_mul(u1v, dzb, wzb)
                        fz = mid_pool.tile([P, CHUNK_F, C], dt.float16,
                                           tag="fz")
                        fzv = fz[:].rearrange("p (m u) e -> p m u e", u=OCT)
                        nc.vector.tensor_add(fzv, f0b, u1v)

                        # q = g2 * fz ; tree-reduce 96 -> 12 ; reduce -> out
                        q = mid_pool.tile([P, CHUNK_F, C], dt.float16,
                                          tag="q")
                        nc.vector.tensor_mul(q[:], g2[:], fz[:])
                        t48 = mid_pool.tile([P, CHUNK_F, 48], dt.float16,
                                            tag="t48")
                        nc.vector.tensor_add(t48[:], q[:, :, 0:48],
                                             q[:, :, 48:96])
                        t24 = mid_pool.tile([P, CHUNK_F, 24], dt.float16,
                                            tag="t24")
                        nc.vector.tensor_add(t24[:], t48[:, :, 0:24],
                                             t48[:, :, 24:48])
                        t12 = mid_pool.tile([P, CHUNK_F, 12], dt.float16,
                                            tag="t12")
                        nc.vector.tensor_add(t12[:], t24[:, :, 0:12],
                                             t24[:, :, 12:24])
                        nc.vector.reduce_sum(
                            out_full[:, CHUNK_F * c:CHUNK_F * (c + 1)],
                            t12[:], axis=Axis.X)

                nc.sync.dma_start(out_d, out_full[:])

    nc.compile()
    return nc


def _build_tables(line_z, line_y, line_x):
    Lz = np.asarray(line_z, dtype=np.float32)
    f0 = Lz.T                                     # (512, 96)
    f1 = np.concatenate([Lz.T[1:], Lz.T[-1:]], axis=0)
    tz = np.zeros((R, EZ), dtype=np.float16)
    tz[:, 0:C] = f0.astype(np.float16)
    tz[:, C:2 * C] = (f1 - f0).astype(np.float16)
    tz[:, 2 * C] = np.arange(R, dtype=np.float16)  # row idx, exact in fp16

    fine = []
    j = np.arange(NJ, dtype=np.float64)
    posj = (j + SC) / Q
    i0 = np.clip(np.floor(posj), 0, R - 1).astype(np.int64)
    i1 = np.clip(i0 + 1, 0, R - 1)
    w = (posj - i0).astype(np.float32)[:, None]
    for L in (line_y, line_x):
        Lf = np.asarray(L, dtype=np.float32).T    # (512, 96)
        t = np.zeros((NJ_PAD, EY), dtype=np.float16)
        t[:NJ, 0:C] = (Lf[i0] * (1.0 - w) + Lf[i1] * w).astype(np.float16)
        fine.append(t)
    return tz, fine[0], fine[1]


def _host_prep(in_tensor, line_z, line_y, line_x):
    """Sort/pack per-core inputs; return (in_maps, orders) for unsharding."""
    global _MAPS
    if _MAPS is None:
        _MAPS = _build_static_maps()
    rank_pf, rank_y16, rank_zo = _MAPS

    pts = np.ascontiguousarray(in_tensor.reshape(-1, 3).astype(np.float32))
    tz, ty, tx = _build_tables(line_z, line_y, line_x)

    in_maps, orders = [], []
    for k in range(N_CORES):
        shard = pts[k * N_CORE:(k + 1) * N_CORE]
        order = np.argsort(shard[:, 2], kind="stable")
        srt = shard[order]                         # sorted by z coord
        pw = np.empty((P, PWCOLS), dtype=np.float32)
        pw[:, ZC0:ZC0 + F] = srt[rank_pf, 2]
        pw[:, Y0:Y0 + F] = srt[rank_y16, 1]
        pw[:, X0:X0 + F] = srt[rank_y16, 0]
        pw[:, ZO0:ZO0 + F // OCT] = srt[rank_zo, 2]
        in_maps.append({"pwa": pw, "tblz": tz, "tbly": ty, "tblx": tx})
        orders.append(order)
    return in_maps, orders


def _unshard(results, orders):
    global _MAPS
    rank_pf = _MAPS[0]
    outs = []
    for k in range(N_CORES):
        w = np.asarray(results[k]["out"])          # [128, 768]
        res_sorted = np.empty(N_CORE, dtype=np.float32)
        res_sorted[rank_pf.reshape(-1)] = w.reshape(-1)
        res = np.empty(N_CORE, dtype=np.float32)
        res[orders[k]] = res_sorted
        outs.append(res)
    return np.concatenate(outs).reshape(4096, 192).astype(np.float32)


def kernel(in_tensor, line_z, line_y, line_x):
    global _BUILT
    from concourse.bass_utils import run_bass_kernel_spmd

    if _BUILT is None:
        _BUILT = _build_nc()
    nc = _BUILT
    in_maps, orders = _host_prep(np.asarray(in_tensor), np.asarray(line_z),
                                 np.asarray(line_y), np.asarray(line_x))
    res = run_bass_kernel_spmd(nc, in_maps, list(range(N_CORES)))
    return _unshard(res.results, orders)


# revision 7
# speedup vs baseline: 1.0007x; 1.0007x over previous
"""CPDecoding (embedding_lookup) Trainium2 kernel, v3.

out[n] = sum_c fz[c,n]*fy[c,n]*fx[c,n], each f* a 1-D linear interpolation
(grid_sample, align_corners=True) of a (96, 512) line table at per-point
coordinates in [0,1).

Strategy (8 cores, data-parallel over N=4096*192 points):
  - Host: shard points; SORT each shard by z-position so consecutive points
    share z table rows; pack coordinates into gather-layouts; build
      * tblz: coarse z table [512, 256] fp16 rows = [f0(96) | delta(96) |
        row_idx | pad] (512B rows),
      * tbly/tblx: fine pre-interpolated tables [32768, 128] fp16 where row
        j = interp(L, (j+32704)/128) (Q=128 sub-steps, 256B rows).
  - Device: octets of 8 z-sorted points share ONE 512B z-row gather
    (8x descriptor sharing); per-point 256B y/x gathers; exact z interp
    fz = f0 + (posz - row_idx)*delta on DVE; fp16 triple product (one mul
    on the Pool engine); pairwise-tree component reduction.
  - Host: inverse-permute the per-core outputs back to input order.

Quantization error (y/x at Q=128 + fp16): rel err ~5.8e-3 (gate 2e-2).
"""

import numpy as np

N_CORES = 8
N_TOTAL = 4096 * 192
N_CORE = N_TOTAL // N_CORES      # 98304 points per core
P = 128                          # partitions
F = N_CORE // P                  # 768 f-columns
C = 96                           # components
R = 512                          # coarse table resolution
Q = 128                          # fine sub-steps per coarse cell (y/x)
SC = (R - 1) / 2 * Q             # 32704: j = round(coord * SC)
NJ = int(SC) + 1                 # 32705 used fine rows
NJ_PAD = 32768                   # padded fine-table rows
EY = 128                         # y/x gather row elems (fp16) = 256B
EZ = 256                         # z gather row elems (fp16) = 512B
GROUPS = 8                       # staging groups (16-partition bands)
CHUNKS_PER_GROUP = 3
N_CHUNKS = GROUPS * CHUNKS_PER_GROUP          # 24
CHUNK_F = F // N_CHUNKS                       # 32 f-cols per chunk
CHUNK_PTS = P * CHUNK_F                       # 4096 points per chunk
OCT = 8                                       # points per shared z-row
OBLK = CHUNK_F // OCT                         # 4 octet blocks per chunk
OCT_G = CHUNKS_PER_GROUP * CHUNK_PTS // OCT   # 1536 octets per group
# combined int16 idx tile columns: [jy (768) | jx (768) | zi (96)]
JY0, JX0, ZI0 = 0, F, 2 * F
JCOLS = 2 * F + F // OCT         # 1632
# combined fp32 input columns: [zc (768) | y16 (768) | x16 (768) | zo16 (96)]
ZC0, Y0, X0, ZO0 = 0, F, 2 * F, 3 * F
PWCOLS = 3 * F + F // OCT        # 2400

_BUILT = None
_MAPS = None


def _build_static_maps():
    """Static slot->rank index maps (no data dependence).

    Processing slot of chunk c: s in [0,4096) -> [p=s%128, f=32c+s//128].
    Octet grouping: df = s//128 = 8m+u; octet o = m*128+p holds sorted ranks
    r = c*4096 + o*8 + u (u=0..7 consecutive in z-sorted order).
    """
    p = np.arange(P)[:, None]
    f = np.arange(F)[None, :]
    c = f // CHUNK_F
    df = f % CHUNK_F
    m = df // OCT
    u = df % OCT
    rank_pf = c * CHUNK_PTS + (m * P + p) * OCT + u          # [128, 768]

    g = np.arange(GROUPS)[:, None, None]
    t = np.arange(16)[None, :, None]
    phi = np.arange(F)[None, None, :]
    sub = phi // 256
    s = (phi % 256) * 16 + t
    pp = s % P
    dff = s // P
    mm = dff // OCT
    uu = dff % OCT
    rank_y16 = ((3 * g + sub) * CHUNK_PTS + (mm * P + pp) * OCT + uu
                ).reshape(GROUPS * 16, F)                     # [128, 768]

    phio = np.arange(F // OCT)[None, None, :]
    og = phio * 16 + t                                        # octet-in-group
    subo = og // (CHUNK_PTS // OCT)
    rem = og % (CHUNK_PTS // OCT)
    rank_zo = ((3 * g + subo) * CHUNK_PTS + rem * OCT + 3
               ).reshape(GROUPS * 16, F // OCT)               # [128, 96]
    return rank_pf, rank_y16, rank_zo


def _build_nc():
    """Build the per-core Bass program (SPMD, identical on all cores)."""
    import concourse.bacc as bacc
    import concourse.tile as tile
    from concourse import mybir
    from concourse.library_config import mlp as lib_mlp

    dt = mybir.dt
    Alu = mybir.AluOpType
    Axis = mybir.AxisListType

    nc = bacc.Bacc("TRN2", target_bir_lowering=False, debug=False,
                   num_devices=N_CORES, num_swdge_queues=1)

    pwa = nc.dram_tensor("pwa", [P, PWCOLS], dt.float32,
                         kind="ExternalInput").ap()
    tblz = nc.dram_tensor("tblz", [R, EZ], dt.float16,
                          kind="ExternalInput").ap()
    tbly = nc.dram_tensor("tbly", [NJ_PAD, EY], dt.float16,
                          kind="ExternalInput").ap()
    tblx = nc.dram_tensor("tblx", [NJ_PAD, EY], dt.float16,
                          kind="ExternalInput").ap()
    out_d = nc.dram_tensor("out", [P, F], dt.float32,
                           kind="ExternalOutput").ap()

    with tile.TileContext(nc) as tc:
        nzo = F // OCT
        with tc.tile_pool(name="persist", bufs=1) as pp:
            posz = pp.tile([P, F], dt.float32, tag="posz")
            jzi = pp.tile([P, nzo], dt.int16, tag="jzi")
            jyx = pp.tile([P, 2 * F], dt.int16, tag="jyx")

            # ---------- setup: load coords, index math ----------
            with tc.tile_pool(name="setup", bufs=1) as sp:
                # z-octet coords first: the first z-gather depends only on
                # the (tiny) zi chain, so it can launch early
                pwz = sp.tile([P, nzo], dt.float32, tag="pwz")
                nc.sync.dma_start(pwz[:], pwa[:, ZO0:ZO0 + nzo])
                pw = sp.tile([P, 3 * F], dt.float32, tag="pw")
                nc.sync.dma_start(pw[:], pwa[:, 0:3 * F])

                def tmp(nm, ncols, dtype=dt.float32):
                    return sp.tile([P, ncols], dtype, tag="tmp", bufs=6,
                                   name=nm)

                def floor_chain(src_ap, dst_ap, ncols, nm, scale, bias,
                                clamp=None):
                    # floor(src*scale + bias) with explicit fixup so the
                    # result is identical whether the fp->int cast truncates
                    # (CoreSim) or rounds to nearest (hardware).
                    jf = tmp(nm, ncols)
                    nc.vector.tensor_scalar(jf[:], src_ap, scale, bias,
                                            Alu.mult, Alu.add)
                    ji = tmp(nm + "i", ncols, dt.int32)
                    nc.vector.tensor_copy(ji[:], jf[:])
                    jb = tmp(nm + "b", ncols)
                    nc.vector.tensor_copy(jb[:], ji[:])
                    jn = tmp(nm + "n", ncols)
                    nc.vector.tensor_tensor(jn[:], jf[:], jb[:], Alu.is_lt)
                    jg = tmp(nm + "g", ncols)
                    nc.vector.tensor_sub(jg[:], jb[:], jn[:])
                    if clamp is not None:
                        jc = tmp(nm + "c", ncols)
                        nc.vector.tensor_scalar(jc[:], jg[:], clamp, 0.0,
                                                Alu.min, Alu.max)
                        jg = jc
                    nc.vector.tensor_copy(dst_ap, jg[:])

                floor_chain(pwz[:], jzi[:], nzo, "zi", 255.5, 255.5,
                            clamp=511.0)
                floor_chain(pw[:, Y0:Y0 + F], jyx[:, 0:F], F, "jy",
                            float(SC), 0.5)
                floor_chain(pw[:, X0:X0 + F], jyx[:, F:2 * F], F, "jx",
                            float(SC), 0.5)

                # posz = zc*255.5 + 255.5  (exact coarse position, fp32)
                nc.vector.tensor_scalar(posz[:], pw[:, ZC0:ZC0 + F],
                                        255.5, 255.5, Alu.mult, Alu.add)

            # ---------- main loop ----------
            GF = CHUNKS_PER_GROUP * CHUNK_F           # 96 f-cols per group
            with (
                tc.tile_pool(name="stgz", bufs=2) as stgz_pool,
                tc.tile_pool(name="stgyx", bufs=2) as stgyx_pool,
                tc.tile_pool(name="zg", bufs=2) as zg_pool,
                tc.tile_pool(name="gath", bufs=3) as gath_pool,
                tc.tile_pool(name="mid", bufs=3) as mid_pool,
                tc.tile_pool(name="og", bufs=2) as og_pool,
            ):
                with tc.tile_critical():
                    nc.gpsimd.load_library(lib_mlp)

                for g in range(GROUPS):
                    # replicate group g's idx rows into every 16-part band
                    stgz = stgz_pool.tile([P, nzo], dt.int16, tag="stgz")
                    srcz = jzi[16 * g:16 * (g + 1), :]
                    for b in range(8):
                        nc.sync.dma_start(stgz[16 * b:16 * (b + 1), :], srcz)
                    stg = stgyx_pool.tile([P, 2 * F], dt.int16, tag="stgyx")
                    src = jyx[16 * g:16 * (g + 1), :]
                    for b in range(8):
                        nc.sync.dma_start(stg[16 * b:16 * (b + 1), :], src)

                    # one z-gather per group: 1536 octet rows of 512B
                    zd = zg_pool.tile([P, OCT_G // P, EZ], dt.float16,
                                      tag="zd")
                    nc.gpsimd.dma_gather(
                        zd[:], tblz, stgz[:], OCT_G, OCT_G,
                        EZ, elem_step=EZ, queue_num=0, single_packet=False)

                    og = og_pool.tile([P, GF], dt.float32, tag="og")

                    for sub in range(CHUNKS_PER_GROUP):
                        c = CHUNKS_PER_GROUP * g + sub
                        gath = []
                        for (tb, col0, nm) in ((tbly, 0, "y"),
                                               (tblx, F, "x")):
                            gt = gath_pool.tile([P, CHUNK_F, EY], dt.float16,
                                                tag=f"g{nm}")
                            idxs = stg[:, col0 + 256 * sub:col0 + 256 * (sub + 1)]
                            nc.gpsimd.dma_gather(
                                gt[:], tb, idxs, CHUNK_PTS, CHUNK_PTS, EY,
                                elem_step=EY, queue_num=0, single_packet=False)
                            gath.append(gt)

                        # z interp first: depends only on the group z-gather,
                        # so only g2/q/tree trail the y/x gather landing
                        # wz = posz - row_idx (row idx baked in z-row elem 192)
                        zrow = zd[:, OBLK * sub:OBLK * (sub + 1), :]
                        i0ap = (zrow[:, :, 2 * C:2 * C + 1]
                                .broadcast_to([P, OBLK, OCT]))
                        pz = (posz[:, CHUNK_F * c:CHUNK_F * (c + 1)]
                              .rearrange("p (m u) -> p m u", u=OCT))
                        wz = mid_pool.tile([P, OBLK, OCT], dt.float16,
                                           tag="wz")
                        nc.vector.tensor_sub(wz[:], pz, i0ap)

                        # fz = f0 + wz*delta
                        wzb = wz[:].unsqueeze(3).broadcast_to(
                            [P, OBLK, OCT, C])
                        dzb = (zrow[:, :, C:2 * C].unsqueeze(2)
                               .broadcast_to([P, OBLK, OCT, C]))
                        f0b = (zrow[:, :, 0:C].unsqueeze(2)
                               .broadcast_to([P, OBLK, OCT, C]))
                        u1 = mid_pool.tile([P, CHUNK_F, C], dt.float16,
                                           tag="u1")
                        u1v = u1[:].rearrange("p (m u) e -> p m u e", u=OCT)
                        nc.vector.tensor_mul(u1v, dzb, wzb)
                        fz = mid_pool.tile([P, CHUNK_F, C], dt.float16,
                                           tag="fz")
                        fzv = fz[:].rearrange("p (m u) e -> p m u e", u=OCT)
                        nc.vector.tensor_add(fzv, f0b, u1v)

                        # g2 = fy*fx ; q = g2*fz ; tree-reduce 96 -> 12
                        g2 = mid_pool.tile([P, CHUNK_F, C], dt.float16,
                                           tag="g2")
                        nc.vector.tensor_mul(g2[:], gath[0][:, :, 0:C],
                                             gath[1][:, :, 0:C])
                        q = mid_pool.tile([P, CHUNK_F, C], dt.float16,
                                          tag="q")
                        nc.vector.tensor_mul(q[:], g2[:], fz[:])
                        t48 = mid_pool.tile([P, CHUNK_F, 48], dt.float16,
                                            tag="t48")
                        nc.vector.tensor_add(t48[:], q[:, :, 0:48],
                                             q[:, :, 48:96])
                        t24 = mid_pool.tile([P, CHUNK_F, 24], dt.float16,
                                            tag="t24")
                        nc.vector.tensor_add(t24[:], t48[:, :, 0:24],
                                             t48[:, :, 24:48])
                        t12 = mid_pool.tile([P, CHUNK_F, 12], dt.float16,
                                            tag="t12")
                        nc.vector.tensor_add(t12[:], t24[:, :, 0:12],
                                             t24[:, :, 12:24])
                        nc.vector.reduce_sum(
                            og[:, CHUNK_F * sub:CHUNK_F * (sub + 1)],
                            t12[:], axis=Axis.X)

                    # store this group's outputs right away
                    nc.sync.dma_start(out_d[:, GF * g:GF * (g + 1)], og[:])

    nc.compile()
    return nc


def _build_tables(line_z, line_y, line_x):
    Lz = np.asarray(line_z, dtype=np.float32)
    f0 = Lz.T                                     # (512, 96)
    f1 = np.concatenate([Lz.T[1:], Lz.T[-1:]], axis=0)
    tz = np.zeros((R, EZ), dtype=np.float16)
    tz[:, 0:C] = f0.astype(np.float16)
    tz[:, C:2 * C] = (f1 - f0).astype(np.float16)
    tz[:, 2 * C] = np.arange(R, dtype=np.float16)  # row idx, exact in fp16

    fine = []
    j = np.arange(NJ, dtype=np.float64)
    posj = (j + SC) / Q
    i0 = np.clip(np.floor(posj), 0, R - 1).astype(np.int64)
    i1 = np.clip(i0 + 1, 0, R - 1)
    w = (posj - i0).astype(np.float32)[:, None]
    for L in (line_y, line_x):
        Lf = np.asarray(L, dtype=np.float32).T    # (512, 96)
        t = np.zeros((NJ_PAD, EY), dtype=np.float16)
        t[:NJ, 0:C] = (Lf[i0] * (1.0 - w) + Lf[i1] * w).astype(np.float16)
        fine.append(t)
    return tz, fine[0], fine[1]


def _host_prep(in_tensor, line_z, line_y, line_x):
    """Sort/pack per-core inputs; return (in_maps, orders) for unsharding."""
    global _MAPS
    if _MAPS is None:
        _MAPS = _build_static_maps()
    rank_pf, rank_y16, rank_zo = _MAPS

    pts = np.ascontiguousarray(in_tensor.reshape(-1, 3).astype(np.float32))
    tz, ty, tx = _build_tables(line_z, line_y, line_x)

    in_maps, orders = [], []
    for k in range(N_CORES):
        shard = pts[k * N_CORE:(k + 1) * N_CORE]
        order = np.argsort(shard[:, 2], kind="stable")
        srt = shard[order]                         # sorted by z coord
        pw = np.empty((P, PWCOLS), dtype=np.float32)
        pw[:, ZC0:ZC0 + F] = srt[rank_pf, 2]
        pw[:, Y0:Y0 + F] = srt[rank_y16, 1]
        pw[:, X0:X0 + F] = srt[rank_y16, 0]
        pw[:, ZO0:ZO0 + F // OCT] = srt[rank_zo, 2]
        in_maps.append({"pwa": pw, "tblz": tz, "tbly": ty, "tblx": tx})
        orders.append(order)
    return in_maps, orders


def _unshard(results, orders):
    global _MAPS
    rank_pf = _MAPS[0]
    outs = []
    for k in range(N_CORES):
        w = np.asarray(results[k]["out"])          # [128, 768]
        res_sorted = np.empty(N_CORE, dtype=np.float32)
        res_sorted[rank_pf.reshape(-1)] = w.reshape(-1)
        res = np.empty(N_CORE, dtype=np.float32)
        res[orders[k]] = res_sorted
        outs.append(res)
    return np.concatenate(outs).reshape(4096, 192).astype(np.float32)


def kernel(in_tensor, line_z, line_y, line_x):
    global _BUILT
    from concourse.bass_utils import run_bass_kernel_spmd

    if _BUILT is None:
        _BUILT = _build_nc()
    nc = _BUILT
    in_maps, orders = _host_prep(np.asarray(in_tensor), np.asarray(line_z),
                                 np.asarray(line_y), np.asarray(line_x))
    res = run_bass_kernel_spmd(nc, in_maps, list(range(N_CORES)))
    return _unshard(res.results, orders)
